# revision 1
# baseline (speedup 1.0000x reference)
"""DecoderAttentionSingle Trainium2 Bass kernel.

8 NeuronCores, pure batch-parallel: one [C,H,W] image per core.

Per core, all channel-major [C, H, W] unless noted (bf16 data, fp32 PSUM):
  k = W_enc^T enc, q = W_dec^T dec          (PE; k stored zero-padded)
  s_n = q + shift_n(k)                       (DVE adds, offsets paired on
  t_n = tanh(s_n + b)                         128 partitions; ACT tanh)
  score_n = W_agg . t_n                      (PE pair-packed dots -> PSUM)
  e_n = exp(score_n + b_agg)                 (ACT, reads PSUM)
  e^T: transpose to pixel-major [W, H, 9]    (PE identity matmuls)
  softmax: mask-mult, reduce, recip, norm    (DVE, pixel-major = cheap)
  attn^T = sum_n w_n * enc^T shifted         (DVE MAC, pixel-major; column
                                              shifts via partition-offset APs)
  vals = conv3x3(dec)                        (PE, 9 accumulated matmuls)
  out = lrelu(W_attn^T [vals; attn] + b)     (PE + ACT drain)
"""

import sys

sys.path.insert(0, "/opt/trn_rl_repo")

from contextlib import ExitStack

import ml_dtypes
import numpy as np

import concourse.bass as bass
import concourse.mybir as mybir
import concourse.tile as tile
from concourse import bacc
from concourse.bass_utils import run_bass_kernel_spmd

BF16 = mybir.dt.bfloat16
FP32 = mybir.dt.float32
AF = mybir.ActivationFunctionType
ALU = mybir.AluOpType

B, ENC, DEC, H, W = 8, 64, 128, 128, 128
HP, WP = H + 2, W + 2
N_CORES = 8

OFFS = [(dr, dc) for dr in (-1, 0, 1) for dc in (-1, 0, 1)]
PAIRS = [(0, 1), (2, 3), (4, 5), (6, 7), (8, None)]

RC = 16          # rows per scores chunk
NCH = H // RC    # 8 chunks
RW = 4           # rows per 512-px matmul window
NW = H // RW     # 32 windows
MRC = 32         # rows per MAC chunk

# packed bf16 constant layout (per-partition element offsets)
OFF_WDEC = 0          # [128, 64]
OFF_WENC = 64         # [64, 64]
OFF_WAGGP = 128       # [128, 2]
OFF_CONVW = 130       # [128, 9*64]
OFF_WATTN = 706       # [128, 64]
OFF_MASKT = 770       # [128(col), 128*9*2] mask replicated x2
OFF_IDENT = 3074      # [128, 128]
OFF_WAGG5 = 3202      # [128, 5*10] sparse pair-dot weights
CONSTB_N = 3252


def build_program():
    nc = bacc.Bacc(None, target_bir_lowering=False, debug=False)

    enc_d = nc.dram_tensor("enc", [ENC, H, W], BF16, kind="ExternalInput").ap()
    dec_d = nc.dram_tensor("dec", [DEC, H, W], BF16, kind="ExternalInput").ap()
    cb_d = nc.dram_tensor("constb", [128, CONSTB_N], BF16, kind="ExternalInput").ap()
    cf_d = nc.dram_tensor("constf", [128, 4], FP32, kind="ExternalInput").ap()
    out_d = nc.dram_tensor("out", [ENC, H, W], BF16, kind="ExternalOutput").ap()

    with tile.TileContext(nc) as tc, ExitStack() as ctx:
        const = ctx.enter_context(tc.tile_pool(name="const", bufs=1))
        big = ctx.enter_context(tc.tile_pool(name="big", bufs=1))
        chunks = ctx.enter_context(tc.tile_pool(name="chunks", bufs=2))

        constb = const.tile([128, CONSTB_N], BF16)
        nc.sync.dma_start(constb[:], cb_d)
        constf = const.tile([128, 4], FP32)
        nc.sync.dma_start(constf[:], cf_d)

        wdec = constb[:, OFF_WDEC:OFF_WDEC + 64]
        wenc = constb[0:64, OFF_WENC:OFF_WENC + 64]
        wagg5 = constb[:, OFF_WAGG5:OFF_WAGG5 + 50].rearrange(
            "p (i m) -> p i m", i=5)
        convw = constb[:, OFF_CONVW:OFF_CONVW + 576].rearrange("p (n c) -> p n c", n=9)
        wattn = constb[:, OFF_WATTN:OFF_WATTN + 64]
        mask2 = constb[:, OFF_MASKT:OFF_MASKT + 2304].rearrange(
            "p (h n a) -> p h n a", n=9, a=2)
        ident = constb[:, OFF_IDENT:OFF_IDENT + 128]
        bb = constf[:, 0:1]        # b_dec + b_enc, stacked twice
        bconv = constf[0:64, 1:2]
        battn = constf[0:64, 2:3]
        bagg9 = constf[0:10, 3:4]

        decp = big.tile([DEC, HP, WP], BF16, tag="decp")
        encs = big.tile([ENC, H, W], BF16, tag="enc_cat")
        kpad = big.tile([ENC, HP, WP], BF16, tag="kpad")
        q = big.tile([ENC, H, W], BF16, tag="q_out")
        ent = big.tile([W, H, ENC], BF16, tag="ent")
        et2 = big.tile([W, H, 9, 2], BF16, tag="et")
        sums = big.tile([W, 2, H], FP32, tag="sums")   # [:,0]=sum, [:,1]=recip

        nc.gpsimd.memset(decp[:, 0, :], 0.0)
        nc.gpsimd.memset(decp[:, HP - 1, :], 0.0)
        nc.gpsimd.memset(decp[:, :, 0:1], 0.0)
        nc.gpsimd.memset(decp[:, :, WP - 1:WP], 0.0)
        nc.gpsimd.memset(kpad[:, 0, :], 0.0)
        nc.gpsimd.memset(kpad[:, HP - 1, :], 0.0)
        nc.gpsimd.memset(kpad[:, :, 0:1], 0.0)
        nc.gpsimd.memset(kpad[:, :, WP - 1:WP], 0.0)

        nc.sync.dma_start(encs[:], enc_d)
        nc.sync.dma_start(decp[:, 1:HP - 1, 1:WP - 1], dec_d)

        # ---------- phase 1: k, q, ent ----------
        with tc.tile_pool(name="psk", bufs=2, space=bass.MemorySpace.PSUM) as psk:
            for g in range(8):
                kp = psk.tile([ENC, 16 * W], FP32, tag="kp")
                for wi in range(4):
                    w = g * 4 + wi
                    nc.tensor.matmul(
                        kp[:, wi * RW * W:(wi + 1) * RW * W], wenc,
                        encs[:, w * RW:(w + 1) * RW, :], start=True, stop=True)
                nc.scalar.activation(
                    kpad[:, 1 + g * 16:1 + (g + 1) * 16, 1:WP - 1],
                    kp[:].rearrange("c (r w) -> c r w", r=16), AF.Copy)
        with tc.tile_pool(name="psq", bufs=2, space=bass.MemorySpace.PSUM) as psq:
            for g in range(8):
                qp = psq.tile([ENC, 16 * W], FP32, tag="qp")
                for wi in range(4):
                    w = g * 4 + wi
                    nc.tensor.matmul(
                        qp[:, wi * RW * W:(wi + 1) * RW * W], wdec,
                        decp[:, 1 + w * RW:1 + (w + 1) * RW, 1:WP - 1],
                        start=True, stop=True)
                nc.scalar.activation(
                    q[:, g * 16:(g + 1) * 16, :],
                    qp[:].rearrange("c (r w) -> c r w", r=16), AF.Copy)
        with tc.tile_pool(name="pse1", bufs=2, space=bass.MemorySpace.PSUM) as pse1:
            for g in range(H // 8):
                ep = pse1.tile([W, 8 * ENC], FP32, tag="ep")
                for j in range(8):
                    nc.tensor.matmul(
                        ep[:, j * ENC:(j + 1) * ENC],
                        encs[:, g * 8 + j, :], ident[0:ENC, 0:ENC],
                        start=True, stop=True)
                nc.vector.tensor_copy(
                    ent[:, g * 8:(g + 1) * 8, :],
                    ep[:].rearrange("p (r c) -> p r c", r=8))

        # ---------- phase 2: scores pipeline ----------
        cat = big.tile([2 * ENC, H, W], BF16, tag="enc_cat")
        entl = big.tile([W, H, ENC], BF16, tag="kpad")
        entr = big.tile([W, H, ENC], BF16, tag="q_out")
        NMC = H // MRC

        def shift_setup(rc):
            r0 = rc * MRC
            nc.gpsimd.memset(entl[96:128, r0:r0 + MRC, :], 0.0)
            nc.gpsimd.memset(entr[0:32, r0:r0 + MRC, :], 0.0)
            nc.sync.dma_start(entl[0:W - 1, r0:r0 + MRC, :],
                              ent[1:W, r0:r0 + MRC, :])
            nc.sync.dma_start(entr[1:W, r0:r0 + MRC, :],
                              ent[0:W - 1, r0:r0 + MRC, :])

        accpm = big.tile([W, H, ENC], BF16, tag="accpm")
        nc.gpsimd.memset(accpm[:, 0, :], 0.0)
        acc4 = accpm.rearrange("p h (a b) -> p h a b", b=2)

        def mac_offsets(offs, r0, nr_rows, first):
            prod = chunks.tile([W, MRC, ENC // 2, 2], BF16, tag="prod")
            for idx, n in enumerate(offs):
                dr, dc = OFFS[n]
                rlo = max(r0, -dr)
                rhi = min(r0 + nr_rows, H - max(0, dr))
                nr = rhi - rlo
                src_ent = entl if dc == 1 else (entr if dc == -1 else ent)
                wsl = et2[:, rlo:rhi, n:n + 1, :].broadcast_to(
                    [W, nr, ENC // 2, 2])
                esl = src_ent[:, rlo + dr:rhi + dr, :].rearrange(
                    "p r (a b) -> p r a b", b=2)
                if first and idx == 0:
                    nc.vector.tensor_tensor(
                        acc4[:, rlo:rhi], wsl, esl, ALU.mult)
                else:
                    nc.vector.tensor_tensor(
                        prod[:, 0:nr], wsl, esl, ALU.mult)
                    nc.vector.tensor_tensor(
                        acc4[:, rlo:rhi], acc4[:, rlo:rhi],
                        prod[:, 0:nr], ALU.add)
        with tc.tile_pool(name="ps2", bufs=1, space=bass.MemorySpace.PSUM) as psd, \
                tc.tile_pool(name="ps3", bufs=2, space=bass.MemorySpace.PSUM) as pse:
            for ch in range(NCH):
                r0 = ch * RC
                dps = psd.tile([10, RC * W], FP32, tag="dps")
                for pi, (n0, n1) in enumerate(PAIRS):
                    spair = chunks.tile([2 * ENC, RC, W], BF16, tag="sp")
                    hi = 2 * ENC if n1 is not None else ENC
                    for half, n in enumerate((n0, n1)):
                        if n is None:
                            continue
                        dr, dc = OFFS[n]
                        lo = half * ENC
                        nc.vector.tensor_tensor(
                            spair[lo:lo + ENC],
                            q[:, r0:r0 + RC, :],
                            kpad[:, 1 + r0 + dr:1 + r0 + RC + dr,
                                 1 + dc:1 + W + dc],
                            ALU.add)
                    nc.scalar.activation(
                        spair[0:hi].rearrange("c r w -> c (r w)"),
                        spair[0:hi].rearrange("c r w -> c (r w)"),
                        AF.Tanh, bias=bb[0:hi])
                    for wi in range(RC // RW):
                        rhs = spair[0:hi, wi * RW:(wi + 1) * RW, :]
                        nc.tensor.matmul(
                            dps[:, wi * RW * W:(wi + 1) * RW * W],
                            wagg5[0:hi, pi, :], rhs,
                            start=(pi == 0), stop=(pi == 4))
                ech = chunks.tile([10, RC * W], BF16, tag="sp")
                nc.scalar.activation(ech[:], dps[:], AF.Exp, bias=bagg9)
                etp = pse.tile([W, RC * 9], FP32, tag="etp")
                ech3 = ech.rearrange("n (r w) -> n r w", r=RC)
                for j in range(RC):
                    nc.tensor.matmul(
                        etp[:, j * 9:(j + 1) * 9],
                        ech3[0:9, j, :], ident[0:9, 0:9],
                        start=True, stop=True)
                # expand x2 into et2, mask, row-sum, recip, normalize
                e2c = et2[:, r0:r0 + RC, :, :]
                nc.vector.tensor_copy(
                    e2c,
                    etp[:].rearrange("p (r n) -> p r n", r=RC)
                    .unsqueeze(3).broadcast_to([W, RC, 9, 2]))
                nc.vector.tensor_tensor(
                    e2c, e2c, mask2[:, r0:r0 + RC, :, :], ALU.mult)
                nc.vector.tensor_reduce(
                    out=sums[:, 0, r0:r0 + RC], in_=e2c,
                    axis=mybir.AxisListType.XY, op=ALU.add)
                nc.vector.reciprocal(
                    sums[:, 1, r0:r0 + RC], sums[:, 0, r0:r0 + RC])
                nc.vector.tensor_tensor(
                    e2c, e2c,
                    sums[:, 1, r0:r0 + RC].unsqueeze(2).unsqueeze(3)
                    .broadcast_to([W, RC, 9, 2]),
                    ALU.mult)
                if ch % 2 == 1:
                    mac_offsets([1, 4, 7], (ch // 2) * MRC, MRC, True)
                # conv for this chunk's windows (PE stays busy; decp dies at p2 end)
                for wi in range(RC // RW):
                    wr0 = r0 + wi * RW
                    cp = pse.tile([ENC, RW * W], FP32, tag="cp")
                    for n, (dr, dc) in enumerate(OFFS):
                        nc.tensor.matmul(
                            cp[:], convw[:, n, :],
                            decp[:, 1 + wr0 + dr:1 + wr0 + RW + dr,
                                 1 + dc:1 + W + dc],
                            start=(n == 0), stop=(n == 8))
                    nc.scalar.activation(
                        cat[0:ENC, wr0:wr0 + RW, :],
                        cp[:].rearrange("c (r w) -> c r w", r=RW),
                        AF.Identity, bias=bconv)

        # ---------- MAC (dc=+-1) + back-transposes + final ----------
        with tc.tile_pool(name="ps5", bufs=2, space=bass.MemorySpace.PSUM) as ps5:
            shift_setup(0)
            for rc_i in range(NMC):
                if rc_i + 1 < NMC:
                    shift_setup(rc_i + 1)
                r0 = rc_i * MRC
                mac_offsets([0, 3, 6, 2, 5, 8], r0, MRC, False)
                # attn rows back to channel-major, then final + leaky + out
                for g in range(r0 // 8, (r0 + MRC) // 8):
                    bp = ps5.tile([ENC, 8 * W], FP32, tag="bp")
                    for j in range(8):
                        nc.tensor.matmul(
                            bp[:, j * W:(j + 1) * W],
                            accpm[:, g * 8 + j, :], ident,
                            start=True, stop=True)
                    if g % 2 == 0:
                        nc.vector.tensor_copy(
                            cat[ENC:, g * 8:(g + 1) * 8, :],
                            bp[:].rearrange("c (r w) -> c r w", r=8))
                    else:
                        nc.scalar.activation(
                            cat[ENC:, g * 8:(g + 1) * 8, :],
                            bp[:].rearrange("c (r w) -> c r w", r=8), AF.Copy)
                    fp = ps5.tile([ENC, 8 * W], FP32, tag="fp")
                    for wi in range(2):
                        nc.tensor.matmul(
                            fp[:, wi * RW * W:(wi + 1) * RW * W], wattn,
                            cat[:, g * 8 + wi * RW:g * 8 + (wi + 1) * RW, :],
                            start=True, stop=True)
                    tll = chunks.tile([ENC, 8, W], BF16, tag="tll")
                    nc.scalar.activation(
                        tll[:], fp[:].rearrange("c (r w) -> c r w", r=8),
                        AF.Identity, bias=battn)
                    ull = chunks.tile([ENC, 8, W], BF16, tag="ull")
                    nc.vector.tensor_single_scalar(ull[:], tll[:], 0.2, ALU.mult)
                    outc = chunks.tile([ENC, 8, W], BF16, tag="outc")
                    nc.vector.tensor_max(outc[:], tll[:], ull[:])
                    nc.sync.dma_start(
                        out_d[:, g * 8:(g + 1) * 8, :], outc[:])

    nc.compile()
    return nc


_PROG = None
_RUN_KWARGS = {}
_LAST_RESULT = None


def _get_prog():
    global _PROG
    if _PROG is None:
        _PROG = build_program()
    return _PROG


def _make_maskt():
    m = np.zeros((W, H, 9), dtype=np.float32)
    for n, (dr, dc) in enumerate(OFFS):
        cv = np.arange(W) + dc
        rv = np.arange(H) + dr
        m[:, :, n] = (((cv >= 0) & (cv < W))[:, None]
                      & ((rv >= 0) & (rv < H))[None, :]).astype(np.float32)
    return m


def _pack_constb(W_dec, W_enc, W_agg, conv_w, W_attn):
    bf = ml_dtypes.bfloat16
    cb = np.zeros((128, CONSTB_N), dtype=np.float32)
    cb[:, OFF_WDEC:OFF_WDEC + 64] = W_dec
    cb[0:64, OFF_WENC:OFF_WENC + 64] = W_enc
    w5 = np.zeros((128, 5, 10), dtype=np.float32)
    for pi in range(4):
        w5[0:64, pi, 2 * pi] = W_agg[:, 0]
        w5[64:128, pi, 2 * pi + 1] = W_agg[:, 0]
    w5[0:64, 4, 8] = W_agg[:, 0]
    cb[:, OFF_WAGG5:OFF_WAGG5 + 50] = w5.reshape(128, 50)
    cw = np.asarray(conv_w, np.float32).reshape(9, DEC, ENC).transpose(1, 0, 2)
    cb[:, OFF_CONVW:OFF_CONVW + 576] = cw.reshape(128, 576)
    wa = np.asarray(W_attn, np.float32).copy()
    wa[ENC:, :] *= 2.0  # compensate exp-weights duplicated (sum = 2S)
    cb[:, OFF_WATTN:OFF_WATTN + 64] = wa
    m2 = np.repeat(_make_maskt().reshape(W, H, 9, 1), 2, axis=3)
    cb[:, OFF_MASKT:OFF_MASKT + 2304] = m2.reshape(128, 2304)
    cb[:, OFF_IDENT:OFF_IDENT + 128] = np.eye(128, dtype=np.float32)
    return cb.astype(bf)


def kernel(encoder_features, decoder_features, W_enc, b_enc, W_dec, b_dec,
           W_agg, b_agg, W_attn, b_attn, conv_w, conv_b):
    bf = ml_dtypes.bfloat16
    nc = _get_prog()

    cf = np.zeros((128, 4), dtype=np.float32)
    bsum = np.asarray(b_dec, np.float32) + np.asarray(b_enc, np.float32)
    cf[0:64, 0] = bsum
    cf[64:128, 0] = bsum
    cf[0:64, 1] = np.asarray(conv_b, np.float32)
    cf[0:64, 2] = np.asarray(b_attn, np.float32)
    cf[0:10, 3] = float(np.asarray(b_agg).reshape(-1)[0])

    shared = {
        "constb": _pack_constb(W_dec, W_enc, np.asarray(W_agg, np.float32),
                               conv_w, W_attn),
        "constf": cf,
    }
    enc_all = np.asarray(encoder_features, np.float32).astype(bf)
    dec_all = np.asarray(decoder_features, np.float32).astype(bf)
    in_maps = []
    for c in range(N_CORES):
        m = dict(shared)
        m["enc"] = enc_all[c]
        m["dec"] = dec_all[c]
        in_maps.append(m)

    res = run_bass_kernel_spmd(nc, in_maps, list(range(N_CORES)),
                               **_RUN_KWARGS)
    global _LAST_RESULT
    _LAST_RESULT = res
    out = np.stack(
        [np.asarray(res.results[c]["out"], np.float32) for c in range(N_CORES)])
    return out

